# revision 8
# baseline (speedup 1.0000x reference)
"""Multi-head attention (RoPE, causal) Trainium2 kernel, 8-way sharded.

Sharding: core c -> (batch b = c//2, head-group g = c%2 of 8 heads).
Each core computes its batch/head-group's attention output projected through
its W_proj row-slice; the host sums the two partial projections per batch and
adds b_proj.

Per-core device pipeline (all matmul inputs fp16, PSUM accumulation fp32):
  1. qT/kT = W'^T @ x^T feature-major [dims, tokens], with W' column-permuted
     on the host so each head's dims are [evens | odds] (RoPE de-interleave).
     q and k interleaved per contraction chunk (dc-outer over 8 PSUM tiles)
     so the gemm streams xT as it arrives from HBM.  RoPE applied via a
     32-partition-block swap (SBUF->SBUF DMA) plus elementwise cos/sin
     tables; PSUM evacuation + bias on VectorE.  v token-major [tokens,dims].
  2. Flash-style attention in S^T layout, software-pipelined: S^T[k,q] tiles
     via row-packed (2 heads) K=64 matmuls; exp on ScalarE from PSUM with the
     1/sqrt(Dh) scale fused; causal masking trims S/exp/PV to the valid
     column range on diagonal tiles and multiplies only the 128-col partial
     band by a 0/1 table; P^T@V (col-packed, denominator via ones rows of
     vaug) lags one k-tile behind S^T so TensorE never waits on exp; gemm /
     projection matmuls for later phases are pulled in as fillers one per
     k-tile to keep TensorE saturated.
  3. out = oT^T @ W_proj_slice per 128-token tile, fp16 DMA to HBM.
"""

import numpy as np

import concourse.bass as bass
import concourse.bacc as bacc
import concourse.tile as tile
import concourse.mybir as mybir
from concourse.bass_utils import run_bass_kernel_spmd

F16 = mybir.dt.float16
F32 = mybir.dt.float32

B, L, D = 4, 2048, 1024
H, Dh = 16, 64
ROPE_THETA = 10000.0
N_CORES = 8
HL = 8           # heads per core
DC = D // 128    # 8 contraction chunks
NJJ = 4          # head pairs per core
NTC = L // 512   # 4 token chunks of 512
NTT = L // 128   # 16 token tiles of 128
NQC = L // 512   # 4 q chunks of 512


def _emit(nc, tc, dram, debug=False):
    """Emit the per-core Tile program."""
    from contextlib import ExitStack

    with ExitStack() as ctx:
        consts = ctx.enter_context(tc.tile_pool(name="consts", bufs=1))
        rope = ctx.enter_context(tc.tile_pool(name="rope", bufs=2))
        ptp = ctx.enter_context(tc.tile_pool(name="ptp", bufs=3))
        nrm = ctx.enter_context(tc.tile_pool(name="nrm", bufs=2))
        ostg = ctx.enter_context(tc.tile_pool(name="ostg", bufs=3))

        # ---- resident tensors -------------------------------------------
        xT = consts.tile([128, DC, L], F16)
        wq = consts.tile([128, DC, 512], F16)
        wk = consts.tile([128, DC, 512], F16)
        wv = consts.tile([128, DC, 512], F16)
        wp = consts.tile([128, NJJ, 1024], F16)
        cos4 = consts.tile([128, L], F16)
        sin4 = consts.tile([128, L], F16)
        masks = consts.tile([128, 4, 512], F16)
        bq = consts.tile([128, NJJ], F32)
        bk = consts.tile([128, NJJ], F32)
        bv = consts.tile([1, 512], F16)
        ones1 = consts.tile([1, 128], F16)
        qT = consts.tile([128, NJJ, L], F16)
        kT = consts.tile([128, NJJ, L], F16)
        vaug = consts.tile([128, NTT, HL, 128], F16)
        oT = consts.tile([128, NJJ, L], F16)

        # ---- input DMA, ordered for the dc-outer q/k gemm ---------------
        nc.sync.dma_start(bq[:], dram["bq"].ap())
        nc.sync.dma_start(bk[:], dram["bk"].ap())
        nc.sync.dma_start(bv[:], dram["bv"].ap())
        nc.sync.dma_start(wq[:, 0, :], dram["wq"].ap()[:, 0, :])
        # split the first x chunk so the very first matmul starts sooner
        for q in range(4):
            nc.sync.dma_start(
                xT[:, 0, q * 512:(q + 1) * 512],
                dram["xT"].ap()[:, 0, q * 512:(q + 1) * 512],
            )
        nc.sync.dma_start(wk[:, 0, :], dram["wk"].ap()[:, 0, :])
        for dc in range(1, DC):
            nc.sync.dma_start(wq[:, dc, :], dram["wq"].ap()[:, dc, :])
            nc.sync.dma_start(xT[:, dc, :], dram["xT"].ap()[:, dc, :])
            nc.sync.dma_start(wk[:, dc, :], dram["wk"].ap()[:, dc, :])
        nc.sync.dma_start(cos4[:], dram["cos4"].ap())
        nc.sync.dma_start(sin4[:], dram["sin4"].ap())
        nc.sync.dma_start(masks[:], dram["masks"].ap())
        for dc in range(DC):
            nc.sync.dma_start(wv[:, dc, :], dram["wv"].ap()[:, dc, :])
        nc.sync.dma_start(wp[:], dram["wp"].ap())
        nc.vector.memset(ones1[:], 1.0)
        nc.gpsimd.memset(vaug[:, :, :, 64:128], 1.0)

        # ---- RoPE store: evac+bias on DVE, swap via SBUF DMA ------------
        def rope_store(ps, b_sb, dstT, jj, ts):
            raw = rope.tile([128, 512], F16, tag="raw")
            nc.vector.tensor_scalar_add(raw[:], ps[:], b_sb[:, jj:jj + 1])
            swp = rope.tile([128, 512], F16, tag="swp")
            for blk in range(4):
                sb_ = (blk ^ 1) * 32
                nc.sync.dma_start(
                    swp[blk * 32:(blk + 1) * 32, :], raw[sb_:sb_ + 32, :]
                )
            t1 = rope.tile([128, 512], F16, tag="t1")
            nc.vector.tensor_mul(t1[:], raw[:], cos4[:, ts:ts + 512])
            t2 = rope.tile([128, 512], F16, tag="t2")
            nc.vector.tensor_mul(t2[:], swp[:], sin4[:, ts:ts + 512])
            nc.gpsimd.tensor_add(dstT[:, jj, ts:ts + 512], t1[:], t2[:])

        # ---- phase 0: q/k gemm for jj=0, dc-outer across 8 PSUM tiles ---
        def qk_gemm_dcouter(jj, pg):
            pss = {}
            for wi, (w_sb, b_sb, dstT) in enumerate(
                ((wq, bq, qT), (wk, bk, kT))
            ):
                for tcn in range(NTC):
                    pss[wi, tcn] = pg.tile(
                        [128, 512], F32, tag=f"g{wi}{tcn}",
                        name=f"g{wi}{tcn}",
                    )
            for dc in range(DC):
                for wi, w_sb in enumerate((wq, wk)):
                    for tcn in range(NTC):
                        nc.tensor.matmul(
                            pss[wi, tcn][:],
                            w_sb[:, dc, jj * 128:(jj + 1) * 128],
                            xT[:, dc, tcn * 512:tcn * 512 + 512],
                            start=(dc == 0),
                            stop=(dc == DC - 1),
                        )
            for tcn in range(NTC):
                rope_store(pss[0, tcn], bq, qT, jj, tcn * 512)
                rope_store(pss[1, tcn], bk, kT, jj, tcn * 512)

        def v_gemm(pg):
            for tt in range(NTT):
                ps = pg.tile([128, 512], F32, tag="gemm")
                for dc in range(DC):
                    nc.tensor.matmul(
                        ps[:],
                        xT[:, dc, tt * 128:(tt + 1) * 128],
                        wv[:, dc, :],
                        start=(dc == 0),
                        stop=False,
                    )
                nc.tensor.matmul(
                    ps[:], ones1[:], bv[:], start=False, stop=True,
                )
                nc.scalar.copy(
                    vaug[:, tt, :, 0:64],
                    ps[:].rearrange("p (h d) -> p h d", h=HL),
                )

        # ---- filler generators: one PE matmul per next() ----------------
        def qk_gen(jj, pg):
            for w_sb, b_sb, dstT in ((wq, bq, qT), (wk, bk, kT)):
                for tcn in range(NTC):
                    ts = tcn * 512
                    ps = pg.tile([128, 512], F32, tag="gemm")
                    for dc in range(DC):
                        nc.tensor.matmul(
                            ps[:],
                            w_sb[:, dc, jj * 128:(jj + 1) * 128],
                            xT[:, dc, ts:ts + 512],
                            start=(dc == 0),
                            stop=(dc == DC - 1),
                        )
                        if dc < DC - 1:
                            yield
                    rope_store(ps, b_sb, dstT, jj, ts)
                    yield

        def proj_gen(qc, pg):
            for tt in range(4 * qc, 4 * qc + 4):
                for cc in range(2):
                    ps = pg.tile([128, 512], F32, tag="gemm")
                    for jj in range(NJJ):
                        nc.tensor.matmul(
                            ps[:],
                            oT[:, jj, tt * 128:(tt + 1) * 128],
                            wp[:, jj, cc * 512:cc * 512 + 512],
                            start=(jj == 0), stop=(jj == NJJ - 1),
                        )
                        if jj < NJJ - 1:
                            yield
                    stage = ostg.tile([128, 512], F16)
                    nc.vector.tensor_copy(stage[:], ps[:])
                    nc.sync.dma_start(
                        dram["out"].ap()[tt * 128:(tt + 1) * 128,
                                         cc * 512:cc * 512 + 512],
                        stage[:],
                    )
                    yield

        fillers = []

        def pull(n=1):
            for _ in range(n):
                while fillers:
                    try:
                        next(fillers[0])
                        break
                    except StopIteration:
                        fillers.pop(0)
                else:
                    return

        def flush():
            while fillers:
                try:
                    next(fillers[0])
                except StopIteration:
                    fillers.pop(0)

        # ---- attention for one (head pair, q chunk), PV lagging by 1 ----
        def attention_block(jj, qc, pp, po):
            qs = qc * 512
            pso = [po.tile([128, 512], F32, tag="psoA", name=f"psoA_{jj}_{qc}"),
                   po.tile([128, 512], F32, tag="psoB", name=f"psoB_{jj}_{qc}")]
            nkt = 4 * qc + 4

            def emit_pv(kt, ptile, c0):
                first, last = kt == 0, kt == nkt - 1
                for h in range(2):
                    nc.tensor.matmul(
                        pso[h][:, c0:512], vaug[:, kt, 2 * jj + h, :],
                        ptile[:, h, c0:512],
                        start=first, stop=last, skip_group_check=True,
                    )

            pending = None
            for kt in range(nkt):
                ks = kt * 128
                diag = kt - 4 * qc
                c0 = 128 * diag if diag > 0 else 0
                pst = pp.tile([128, 2, 512], F32, tag="pst")
                for h in range(2):
                    nc.tensor.matmul(
                        pst[0:128, h, c0:512],
                        kT[64 * h:64 * h + 64, jj, ks:ks + 128],
                        qT[64 * h:64 * h + 64, jj, qs + c0:qs + 512],
                        start=True, stop=True,
                    )
                ptile = ptp.tile([128, 2, 512], F16, tag="ptile")
                nc.scalar.activation(
                    ptile[:, :, c0:512], pst[:, :, c0:512],
                    mybir.ActivationFunctionType.Exp, scale=0.125,
                )
                if diag >= 0:
                    bhi = min(c0 + 128, 512)
                    for h in range(2):
                        nc.vector.tensor_mul(
                            ptile[:, h, c0:bhi], ptile[:, h, c0:bhi],
                            masks[:, diag, c0:bhi],
                        )
                pull(1)
                if pending is not None:
                    emit_pv(*pending)
                pending = (kt, ptile, c0)
            emit_pv(*pending)
            # normalize: copy PSO out fast (frees the bank), then
            # oT rows = num * recip(den)
            for h in range(2):
                cpn = nrm.tile([64, 512], F32, tag=f"cpn{h}")
                nc.vector.tensor_copy(cpn[:], pso[h][0:64, :])
                cpd = nrm.tile([64, 512], F32, tag=f"cpd{h}")
                nc.vector.tensor_copy(cpd[:], pso[h][64:128, :])
                rec = nrm.tile([64, 512], F32, tag=f"rec{h}")
                nc.vector.reciprocal_approx_fast(rec[:], cpd[:])
                nc.vector.tensor_mul(
                    oT[64 * h:64 * h + 64, jj, qs:qs + 512],
                    cpn[:], rec[:],
                )

        with tc.tile_pool(name="pgA", bufs=1, space="PSUM") as pga:
            qk_gemm_dcouter(0, pga)
        with tc.tile_pool(name="pgV", bufs=4, space="PSUM") as pgv:
            v_gemm(pgv)
        with (
            tc.tile_pool(name="pg", bufs=2, space="PSUM") as pg,
            tc.tile_pool(name="pp", bufs=2, space="PSUM") as pp,
            tc.tile_pool(name="po", bufs=1, space="PSUM") as po,
        ):
            for jj in range(NJJ):
                if jj < NJJ - 1:
                    fillers.append(qk_gen(jj + 1, pg))
                for qc in range(NQC):
                    attention_block(jj, qc, pp, po)
                    if jj == NJJ - 1:
                        fillers.append(proj_gen(qc, pg))
                if jj < NJJ - 1:
                    flush()
            flush()

        if debug:
            nc.sync.dma_start(dram["dbg_qT"].ap(), qT[:])
            nc.sync.dma_start(dram["dbg_kT"].ap(), kT[:])
            nc.sync.dma_start(dram["dbg_vaug"].ap(), vaug[:])
            nc.sync.dma_start(dram["dbg_oT"].ap(), oT[:])


def build(debug=False):
    nc = bacc.Bacc("TRN2", target_bir_lowering=False, debug=False)
    dram = {
        "xT": nc.dram_tensor("xT", [128, DC, L], F16, kind="ExternalInput"),
        "wq": nc.dram_tensor("wq", [128, DC, 512], F16, kind="ExternalInput"),
        "wk": nc.dram_tensor("wk", [128, DC, 512], F16, kind="ExternalInput"),
        "wv": nc.dram_tensor("wv", [128, DC, 512], F16, kind="ExternalInput"),
        "wp": nc.dram_tensor("wp", [128, NJJ, 1024], F16, kind="ExternalInput"),
        "cos4": nc.dram_tensor("cos4", [128, L], F16, kind="ExternalInput"),
        "sin4": nc.dram_tensor("sin4", [128, L], F16, kind="ExternalInput"),
        "masks": nc.dram_tensor("masks", [128, 4, 512], F16, kind="ExternalInput"),
        "bq": nc.dram_tensor("bq", [128, NJJ], F32, kind="ExternalInput"),
        "bk": nc.dram_tensor("bk", [128, NJJ], F32, kind="ExternalInput"),
        "bv": nc.dram_tensor("bv", [1, 512], F16, kind="ExternalInput"),
        "out": nc.dram_tensor("out", [L, D], F16, kind="ExternalOutput"),
    }
    if debug:
        dram["dbg_qT"] = nc.dram_tensor("dbg_qT", [128, NJJ, L], F16, kind="ExternalOutput")
        dram["dbg_kT"] = nc.dram_tensor("dbg_kT", [128, NJJ, L], F16, kind="ExternalOutput")
        dram["dbg_vaug"] = nc.dram_tensor("dbg_vaug", [128, NTT, HL, 128], F16, kind="ExternalOutput")
        dram["dbg_oT"] = nc.dram_tensor("dbg_oT", [128, NJJ, L], F16, kind="ExternalOutput")
    with tile.TileContext(nc) as tc:
        _emit(nc, tc, dram, debug=debug)
    nc.compile()
    return nc


def host_inputs(x, W_qkv, b_qkv, W_proj):
    """Build the 8 per-core input maps (numpy, fp16-cast, pre-laid-out)."""
    x = np.asarray(x, np.float32)
    W_qkv = np.asarray(W_qkv, np.float32)
    b_qkv = np.asarray(b_qkv, np.float32)
    W_proj = np.asarray(W_proj, np.float32)

    # RoPE tables in the de-interleaved (evens|odds) per-32-block layout.
    inv_freq = 1.0 / (ROPE_THETA ** (np.arange(0, Dh, 2, dtype=np.float32) / Dh))
    t = np.arange(L, dtype=np.float32)
    freqs = np.outer(t, inv_freq)            # [L, 32]
    cosT = np.cos(freqs).T.astype(np.float32)  # [32, L]
    sinT = np.sin(freqs).T.astype(np.float32)
    cos4 = np.tile(cosT, (4, 1)).astype(np.float16)           # [128, L]
    sin4 = np.concatenate([-sinT, sinT, -sinT, sinT], 0).astype(np.float16)

    # causal masks for the 4 diagonal offsets: keep iff q >= k + 128*i
    kk = np.arange(128)[:, None]
    qq = np.arange(512)[None, :]
    masks = np.stack(
        [(qq >= kk + 128 * i) for i in range(4)], axis=1
    ).astype(np.float16)                     # [128, 4, 512]

    perm = np.concatenate([np.arange(0, Dh, 2), np.arange(1, Dh, 2)])  # evens|odds

    in_maps = []
    for c in range(N_CORES):
        b, g = c // 2, c % 2
        heads = np.arange(g * HL, g * HL + HL)
        qk_cols = np.concatenate([h * Dh + perm for h in heads])       # [512]
        v_lo = 2 * D + g * 512

        xT = np.ascontiguousarray(x[b].T)                  # [D, L]
        xT = xT.reshape(DC, 128, L).transpose(1, 0, 2)     # [128, DC, L]

        def wslice(cols_base, cols):
            w = W_qkv[:, cols_base + cols] if cols is not None \
                else W_qkv[:, cols_base:cols_base + 512]
            return np.ascontiguousarray(
                w.reshape(DC, 128, 512).transpose(1, 0, 2)).astype(np.float16)

        wq_h = wslice(0, qk_cols)
        wk_h = wslice(D, qk_cols)
        wv_h = wslice(v_lo, None)
        wp_h = np.ascontiguousarray(
            W_proj[g * 512:(g + 1) * 512, :]
            .reshape(NJJ, 128, 1024).transpose(1, 0, 2)).astype(np.float16)
        bq_h = np.ascontiguousarray(
            b_qkv[qk_cols].reshape(NJJ, 128).T).astype(np.float32)
        bk_h = np.ascontiguousarray(
            b_qkv[D + qk_cols].reshape(NJJ, 128).T).astype(np.float32)
        bv_h = b_qkv[v_lo:v_lo + 512].reshape(1, 512).astype(np.float16)

        in_maps.append({
            "xT": xT.astype(np.float16),
            "wq": wq_h, "wk": wk_h, "wv": wv_h, "wp": wp_h,
            "cos4": cos4, "sin4": sin4, "masks": masks,
            "bq": bq_h, "bk": bk_h, "bv": bv_h,
        })
    return in_maps


_NC = None


def kernel(x, W_qkv, b_qkv, W_proj, b_proj, attention_mask):
    global _NC
    if _NC is None:
        _NC = build()
    in_maps = host_inputs(x, W_qkv, b_qkv, W_proj)
    res = run_bass_kernel_spmd(_NC, in_maps, core_ids=list(range(N_CORES)))
    b_proj = np.asarray(b_proj, np.float32)
    out = np.empty((B, L, D), np.float32)
    for b in range(B):
        out[b] = (res.results[2 * b]["out"].astype(np.float32)
                  + res.results[2 * b + 1]["out"].astype(np.float32)
                  + b_proj)
    return out


# revision 12
# speedup vs baseline: 1.2006x; 1.2006x over previous
"""Multi-head attention (RoPE, causal) Trainium2 kernel, 8-way sharded.

Sharding: core c -> (batch b = c//2, head-group g = c%2 of 8 heads).
Each core computes its batch/head-group's attention output projected through
its W_proj row-slice; the host sums the two partial projections per batch and
adds b_proj.

Per-core device pipeline (all matmul inputs fp16, PSUM accumulation fp32):
  1. qT/kT = W'^T @ x^T feature-major [dims, tokens], with W' column-permuted
     on the host so each head's dims are [evens | odds] (RoPE de-interleave).
     q and k interleaved per contraction chunk (dc-outer over 8 PSUM tiles)
     so the gemm streams xT as it arrives from HBM.  RoPE applied via a
     32-partition-block swap (SBUF->SBUF DMA) plus elementwise cos/sin
     tables; PSUM evacuation + bias on VectorE.  v token-major [tokens,dims].
  2. Flash-style attention in S^T layout, software-pipelined: S^T[k,q] tiles
     via row-packed (2 heads) K=64 matmuls; exp on ScalarE from PSUM with the
     1/sqrt(Dh) scale fused; causal masking trims S/exp/PV to the valid
     column range on diagonal tiles and multiplies only the 128-col partial
     band by a 0/1 table; P^T@V (col-packed, denominator via ones rows of
     vaug) lags one k-tile behind S^T so TensorE never waits on exp; gemm /
     projection matmuls for later phases are pulled in as fillers one per
     k-tile to keep TensorE saturated.
  3. out = oT^T @ W_proj_slice per 128-token tile, fp16 DMA to HBM.
"""

import numpy as np

import concourse.bass as bass
import concourse.bacc as bacc
import concourse.tile as tile
import concourse.mybir as mybir
from concourse.bass_utils import run_bass_kernel_spmd

F16 = mybir.dt.float16
F32 = mybir.dt.float32

B, L, D = 4, 2048, 1024
H, Dh = 16, 64
ROPE_THETA = 10000.0
N_CORES = 8
HL = 8           # heads per core
DC = D // 128    # 8 contraction chunks
NJJ = 4          # head pairs per core
NTC = L // 512   # 4 token chunks of 512
NTT = L // 128   # 16 token tiles of 128
NQC = L // 512   # 4 q chunks of 512


def _emit(nc, tc, dram, debug=False):
    """Emit the per-core Tile program."""
    from contextlib import ExitStack

    with ExitStack() as ctx:
        consts = ctx.enter_context(tc.tile_pool(name="consts", bufs=1))
        rope = ctx.enter_context(tc.tile_pool(name="rope", bufs=2))
        ptp = ctx.enter_context(tc.tile_pool(name="ptp", bufs=3))
        nrm = ctx.enter_context(tc.tile_pool(name="nrm", bufs=2))
        ostg = ctx.enter_context(tc.tile_pool(name="ostg", bufs=3))

        # ---- resident tensors -------------------------------------------
        xT = consts.tile([128, DC, L], F16)
        wq = consts.tile([128, DC, 512], F16)
        wk = consts.tile([128, DC, 512], F16)
        wv = consts.tile([128, DC, 512], F16)
        wp = consts.tile([128, NJJ, 1024], F16)
        cos4 = consts.tile([128, L], F16)
        sin4 = consts.tile([128, L], F16)
        masks = consts.tile([128, 4, 512], F16)
        bq = consts.tile([128, NJJ], F32)
        bk = consts.tile([128, NJJ], F32)
        bv = consts.tile([1, 512], F16)
        ones1 = consts.tile([1, 128], F16)
        qT = consts.tile([128, NJJ, L], F16)
        kT = consts.tile([128, NJJ, L], F16)
        vaug = consts.tile([128, NTT, HL, 128], F16)
        oT = consts.tile([128, NJJ, L], F16)

        # ---- input DMA: few large triggers (sync issue is ~0.7us each),
        # ordered so the dc-outer q/k gemm streams xT as it lands ---------
        nc.sync.dma_start(bq[:], dram["bq"].ap())
        nc.sync.dma_start(bk[:], dram["bk"].ap())
        nc.sync.dma_start(bv[:], dram["bv"].ap())
        nc.sync.dma_start(wq[:], dram["wq"].ap())
        nc.sync.dma_start(xT[:, 0:2, :], dram["xT"].ap()[:, 0:2, :])
        nc.sync.dma_start(xT[:, 2:4, :], dram["xT"].ap()[:, 2:4, :])
        nc.sync.dma_start(wk[:], dram["wk"].ap())
        nc.sync.dma_start(xT[:, 4:6, :], dram["xT"].ap()[:, 4:6, :])
        nc.sync.dma_start(xT[:, 6:8, :], dram["xT"].ap()[:, 6:8, :])
        nc.sync.dma_start(wv[:], dram["wv"].ap())
        nc.sync.dma_start(cos4[:], dram["cos4"].ap())
        nc.sync.dma_start(sin4[:], dram["sin4"].ap())
        nc.sync.dma_start(masks[:], dram["masks"].ap())
        nc.sync.dma_start(wp[:], dram["wp"].ap())
        nc.vector.memset(ones1[:], 1.0)
        nc.gpsimd.memset(vaug[:, :, :, 64:128], 1.0)

        # ---- RoPE store: evac+bias on DVE, swap via SBUF DMA ------------
        def rope_store(ps, b_sb, dstT, jj, ts):
            raw = rope.tile([128, 512], F16, tag="raw")
            nc.vector.tensor_scalar_add(raw[:], ps[:], b_sb[:, jj:jj + 1])
            swp = rope.tile([128, 512], F16, tag="swp")
            for blk in range(4):
                sb_ = (blk ^ 1) * 32
                nc.sync.dma_start(
                    swp[blk * 32:(blk + 1) * 32, :], raw[sb_:sb_ + 32, :]
                )
            t1 = rope.tile([128, 512], F16, tag="t1")
            nc.vector.tensor_mul(t1[:], raw[:], cos4[:, ts:ts + 512])
            t2 = rope.tile([128, 512], F16, tag="t2")
            nc.vector.tensor_mul(t2[:], swp[:], sin4[:, ts:ts + 512])
            nc.gpsimd.tensor_add(dstT[:, jj, ts:ts + 512], t1[:], t2[:])

        # ---- phase 0: q/k gemm for jj=0, dc-outer across 8 PSUM tiles ---
        def qk_gemm_dcouter(jj, pg):
            pss = {}
            for wi, (w_sb, b_sb, dstT) in enumerate(
                ((wq, bq, qT), (wk, bk, kT))
            ):
                for tcn in range(NTC):
                    pss[wi, tcn] = pg.tile(
                        [128, 512], F32, tag=f"g{wi}{tcn}",
                        name=f"g{wi}{tcn}",
                    )
            # q over dc 0-3 first (wk lands after xT chunks 0-3), then k
            # catches up, then strict q/k interleave tracks the xT stream.
            sched = ([(0, dc) for dc in range(4)] + [(1, dc) for dc in range(4)]
                     + [(wi, dc) for dc in range(4, DC) for wi in (0, 1)])
            for wi, dc in sched:
                w_sb = (wq, wk)[wi]
                for tcn in range(NTC):
                    nc.tensor.matmul(
                        pss[wi, tcn][:],
                        w_sb[:, dc, jj * 128:(jj + 1) * 128],
                        xT[:, dc, tcn * 512:tcn * 512 + 512],
                        start=(dc == 0),
                        stop=(dc == DC - 1),
                    )
            for tcn in range(NTC):
                rope_store(pss[0, tcn], bq, qT, jj, tcn * 512)
                rope_store(pss[1, tcn], bk, kT, jj, tcn * 512)

        def v_gemm(pg):
            for tt in range(NTT):
                ps = pg.tile([128, 512], F32, tag="gemm")
                for dc in range(DC):
                    nc.tensor.matmul(
                        ps[:],
                        xT[:, dc, tt * 128:(tt + 1) * 128],
                        wv[:, dc, :],
                        start=(dc == 0),
                        stop=False,
                    )
                nc.tensor.matmul(
                    ps[:], ones1[:], bv[:], start=False, stop=True,
                )
                nc.scalar.copy(
                    vaug[:, tt, :, 0:64],
                    ps[:].rearrange("p (h d) -> p h d", h=HL),
                )

        # ---- filler generators: one PE matmul per next() ----------------
        def qk_gen(jj, pg):
            for w_sb, b_sb, dstT in ((wq, bq, qT), (wk, bk, kT)):
                for tcn in range(NTC):
                    ts = tcn * 512
                    ps = pg.tile([128, 512], F32, tag="gemm")
                    for dc in range(DC):
                        nc.tensor.matmul(
                            ps[:],
                            w_sb[:, dc, jj * 128:(jj + 1) * 128],
                            xT[:, dc, ts:ts + 512],
                            start=(dc == 0),
                            stop=(dc == DC - 1),
                        )
                        if dc < DC - 1:
                            yield
                    rope_store(ps, b_sb, dstT, jj, ts)
                    yield

        def proj_gen(qc, pg):
            for tt in range(4 * qc, 4 * qc + 4):
                stage = ostg.tile([128, 1024], F16, tag="stage",
                                  name=f"stage_{tt}")
                for cc in range(2):
                    ps = pg.tile([128, 512], F32, tag="gemm")
                    for jj in range(NJJ):
                        nc.tensor.matmul(
                            ps[:],
                            oT[:, jj, tt * 128:(tt + 1) * 128],
                            wp[:, jj, cc * 512:cc * 512 + 512],
                            start=(jj == 0), stop=(jj == NJJ - 1),
                        )
                        if jj < NJJ - 1:
                            yield
                    nc.vector.tensor_copy(
                        stage[:, cc * 512:cc * 512 + 512], ps[:]
                    )
                    yield
                nc.sync.dma_start(
                    dram["out"].ap()[tt * 128:(tt + 1) * 128, :], stage[:],
                )

        fillers = []

        def pull(n=1):
            for _ in range(n):
                while fillers:
                    try:
                        next(fillers[0])
                        break
                    except StopIteration:
                        fillers.pop(0)
                else:
                    return

        def flush():
            while fillers:
                try:
                    next(fillers[0])
                except StopIteration:
                    fillers.pop(0)

        # ---- attention for one (head pair, q chunk), PV lagging by 1 ----
        def attention_block(jj, qc, pp, po):
            qs = qc * 512
            pso = [po.tile([128, 512], F32, tag="psoA", name=f"psoA_{jj}_{qc}"),
                   po.tile([128, 512], F32, tag="psoB", name=f"psoB_{jj}_{qc}")]
            nkt = 4 * qc + 4

            def emit_pv(kt, ptile, c0):
                first, last = kt == 0, kt == nkt - 1
                for h in range(2):
                    nc.tensor.matmul(
                        pso[h][:, c0:512], vaug[:, kt, 2 * jj + h, :],
                        ptile[:, h, c0:512],
                        start=first, stop=last, skip_group_check=True,
                    )

            pending = None
            for kt in range(nkt):
                ks = kt * 128
                diag = kt - 4 * qc
                c0 = 128 * diag if diag > 0 else 0
                pst = pp.tile([128, 2, 512], F32, tag="pst")
                for h in range(2):
                    nc.tensor.matmul(
                        pst[0:128, h, c0:512],
                        kT[64 * h:64 * h + 64, jj, ks:ks + 128],
                        qT[64 * h:64 * h + 64, jj, qs + c0:qs + 512],
                        start=True, stop=True,
                    )
                ptile = ptp.tile([128, 2, 512], F16, tag="ptile")
                nc.scalar.activation(
                    ptile[:, :, c0:512], pst[:, :, c0:512],
                    mybir.ActivationFunctionType.Exp, scale=0.125,
                )
                if diag >= 0:
                    bhi = min(c0 + 128, 512)
                    for h in range(2):
                        nc.vector.tensor_mul(
                            ptile[:, h, c0:bhi], ptile[:, h, c0:bhi],
                            masks[:, diag, c0:bhi],
                        )
                pull(1)
                if pending is not None:
                    emit_pv(*pending)
                pending = (kt, ptile, c0)
            emit_pv(*pending)
            # normalize: copy PSO out fast (frees the bank), then
            # oT rows = num * recip(den)
            for h in range(2):
                cpn = nrm.tile([64, 512], F32, tag=f"cpn{h}")
                nc.vector.tensor_copy(cpn[:], pso[h][0:64, :])
                cpd = nrm.tile([64, 512], F32, tag=f"cpd{h}")
                nc.vector.tensor_copy(cpd[:], pso[h][64:128, :])
                rec = nrm.tile([64, 512], F32, tag=f"rec{h}")
                nc.vector.reciprocal_approx_fast(rec[:], cpd[:])
                nc.vector.tensor_mul(
                    oT[64 * h:64 * h + 64, jj, qs:qs + 512],
                    cpn[:], rec[:],
                )

        with tc.tile_pool(name="pgA", bufs=1, space="PSUM") as pga:
            qk_gemm_dcouter(0, pga)
        with tc.tile_pool(name="pgV", bufs=4, space="PSUM") as pgv:
            v_gemm(pgv)
        with (
            tc.tile_pool(name="pg", bufs=2, space="PSUM") as pg,
            tc.tile_pool(name="pp", bufs=2, space="PSUM") as pp,
            tc.tile_pool(name="po", bufs=1, space="PSUM") as po,
        ):
            for jj in range(NJJ):
                if jj < NJJ - 1:
                    fillers.append(qk_gen(jj + 1, pg))
                for qc in range(NQC):
                    attention_block(jj, qc, pp, po)
                    if jj == NJJ - 1:
                        fillers.append(proj_gen(qc, pg))
                if jj < NJJ - 1:
                    flush()
            flush()

        if debug:
            nc.sync.dma_start(dram["dbg_qT"].ap(), qT[:])
            nc.sync.dma_start(dram["dbg_kT"].ap(), kT[:])
            nc.sync.dma_start(dram["dbg_vaug"].ap(), vaug[:])
            nc.sync.dma_start(dram["dbg_oT"].ap(), oT[:])


def build(debug=False):
    nc = bacc.Bacc("TRN2", target_bir_lowering=False, debug=False)
    dram = {
        "xT": nc.dram_tensor("xT", [128, DC, L], F16, kind="ExternalInput"),
        "wq": nc.dram_tensor("wq", [128, DC, 512], F16, kind="ExternalInput"),
        "wk": nc.dram_tensor("wk", [128, DC, 512], F16, kind="ExternalInput"),
        "wv": nc.dram_tensor("wv", [128, DC, 512], F16, kind="ExternalInput"),
        "wp": nc.dram_tensor("wp", [128, NJJ, 1024], F16, kind="ExternalInput"),
        "cos4": nc.dram_tensor("cos4", [128, L], F16, kind="ExternalInput"),
        "sin4": nc.dram_tensor("sin4", [128, L], F16, kind="ExternalInput"),
        "masks": nc.dram_tensor("masks", [128, 4, 512], F16, kind="ExternalInput"),
        "bq": nc.dram_tensor("bq", [128, NJJ], F32, kind="ExternalInput"),
        "bk": nc.dram_tensor("bk", [128, NJJ], F32, kind="ExternalInput"),
        "bv": nc.dram_tensor("bv", [1, 512], F16, kind="ExternalInput"),
        "out": nc.dram_tensor("out", [L, D], F16, kind="ExternalOutput"),
    }
    if debug:
        dram["dbg_qT"] = nc.dram_tensor("dbg_qT", [128, NJJ, L], F16, kind="ExternalOutput")
        dram["dbg_kT"] = nc.dram_tensor("dbg_kT", [128, NJJ, L], F16, kind="ExternalOutput")
        dram["dbg_vaug"] = nc.dram_tensor("dbg_vaug", [128, NTT, HL, 128], F16, kind="ExternalOutput")
        dram["dbg_oT"] = nc.dram_tensor("dbg_oT", [128, NJJ, L], F16, kind="ExternalOutput")
    with tile.TileContext(nc) as tc:
        _emit(nc, tc, dram, debug=debug)
    nc.compile()
    return nc


def host_inputs(x, W_qkv, b_qkv, W_proj):
    """Build the 8 per-core input maps (numpy, fp16-cast, pre-laid-out)."""
    x = np.asarray(x, np.float32)
    W_qkv = np.asarray(W_qkv, np.float32)
    b_qkv = np.asarray(b_qkv, np.float32)
    W_proj = np.asarray(W_proj, np.float32)

    # RoPE tables in the de-interleaved (evens|odds) per-32-block layout.
    inv_freq = 1.0 / (ROPE_THETA ** (np.arange(0, Dh, 2, dtype=np.float32) / Dh))
    t = np.arange(L, dtype=np.float32)
    freqs = np.outer(t, inv_freq)            # [L, 32]
    cosT = np.cos(freqs).T.astype(np.float32)  # [32, L]
    sinT = np.sin(freqs).T.astype(np.float32)
    cos4 = np.tile(cosT, (4, 1)).astype(np.float16)           # [128, L]
    sin4 = np.concatenate([-sinT, sinT, -sinT, sinT], 0).astype(np.float16)

    # causal masks for the 4 diagonal offsets: keep iff q >= k + 128*i
    kk = np.arange(128)[:, None]
    qq = np.arange(512)[None, :]
    masks = np.stack(
        [(qq >= kk + 128 * i) for i in range(4)], axis=1
    ).astype(np.float16)                     # [128, 4, 512]

    perm = np.concatenate([np.arange(0, Dh, 2), np.arange(1, Dh, 2)])  # evens|odds

    in_maps = []
    for c in range(N_CORES):
        b, g = c // 2, c % 2
        heads = np.arange(g * HL, g * HL + HL)
        qk_cols = np.concatenate([h * Dh + perm for h in heads])       # [512]
        v_lo = 2 * D + g * 512

        xT = np.ascontiguousarray(x[b].T)                  # [D, L]
        xT = xT.reshape(DC, 128, L).transpose(1, 0, 2)     # [128, DC, L]

        def wslice(cols_base, cols):
            w = W_qkv[:, cols_base + cols] if cols is not None \
                else W_qkv[:, cols_base:cols_base + 512]
            return np.ascontiguousarray(
                w.reshape(DC, 128, 512).transpose(1, 0, 2)).astype(np.float16)

        wq_h = wslice(0, qk_cols)
        wk_h = wslice(D, qk_cols)
        wv_h = wslice(v_lo, None)
        wp_h = np.ascontiguousarray(
            W_proj[g * 512:(g + 1) * 512, :]
            .reshape(NJJ, 128, 1024).transpose(1, 0, 2)).astype(np.float16)
        bq_h = np.ascontiguousarray(
            b_qkv[qk_cols].reshape(NJJ, 128).T).astype(np.float32)
        bk_h = np.ascontiguousarray(
            b_qkv[D + qk_cols].reshape(NJJ, 128).T).astype(np.float32)
        bv_h = b_qkv[v_lo:v_lo + 512].reshape(1, 512).astype(np.float16)

        in_maps.append({
            "xT": xT.astype(np.float16),
            "wq": wq_h, "wk": wk_h, "wv": wv_h, "wp": wp_h,
            "cos4": cos4, "sin4": sin4, "masks": masks,
            "bq": bq_h, "bk": bk_h, "bv": bv_h,
        })
    return in_maps


_NC = None


def kernel(x, W_qkv, b_qkv, W_proj, b_proj, attention_mask):
    global _NC
    if _NC is None:
        _NC = build()
    in_maps = host_inputs(x, W_qkv, b_qkv, W_proj)
    res = run_bass_kernel_spmd(_NC, in_maps, core_ids=list(range(N_CORES)))
    b_proj = np.asarray(b_proj, np.float32)
    out = np.empty((B, L, D), np.float32)
    for b in range(B):
        out[b] = (res.results[2 * b]["out"].astype(np.float32)
                  + res.results[2 * b + 1]["out"].astype(np.float32)
                  + b_proj)
    return out
